# revision 72
# baseline (speedup 1.0000x reference)
"""MinGRU block (RMSNorm -> minGRU scan -> residual -> RMSNorm -> SwiGLU FFN
-> residual) for Trainium2, SPMD over 8 NeuronCores.

Sharding: core c handles batch b=c//2, token-half s=c%2 (2048 tokens each).
Phase 1 (gate/cand matmuls + scan) runs only on the core's own tokens; the
cross-half scan dependency is resolved with the linear-recurrence split
h_true = h_loc + A_loc * carry, where A_loc is the running product of gates
(a second tensor_tensor_scan) and carry = partner's last h, exchanged via a
2KB pairwise AllGather (DRAM bounce). s=0 cores multiply the carry by a 0.0
mask input. The f32 residual spill happens pre-fix; the A*carry term is
re-added to y during phase 2 (bf16 A/carry: the term decays to 0 in ~100
tokens, so bf16 noise on it is negligible).

The FFN runs entirely in fp8e4 (DoubleRow perf mode, 2 k-tiles per PE
instruction = 2x bf16 throughput): weights are pre-scaled by S_W host-side,
the norm output is quantized to fp8 with S_X folded into the rinv broadcast,
and silu(z1)*z3 is quantized to fp8 by the same DVE op that computes it.
All scales are powers of two; dequant folds into the activation scale and
the final residual scalar_tensor_tensor.

Everything on-device is feature-major [D, tokens]: matmuls keep weights
stationary (lhsT tiles [K=128, M=128]) with activations as the moving
operand. RMSNorm's partition-dim reduce/broadcast go through the tensor
engine (ones-vector matmuls); squares run on ScalarE; the residual adds run
on GpSimd; the two scans (h and A), the gate-combine STT, and the bf16
handoff copies run on the DVE. The carry bounce DMAs go through the GpSimd
queue so they don't wait behind weight prefetches on the sync DMA rings, and
a warmup AllGather at program start pays the channel's first-touch cost off
the critical path.
"""

import os
import sys

sys.path.insert(0, "/opt/trn_rl_repo")

from contextlib import ExitStack

import ml_dtypes
import numpy as np

import concourse.bass as bass
import concourse.mybir as mybir
from concourse import bacc
from concourse.tile import TileContext

P = 128
EPS = 1e-6
F32 = mybir.dt.float32
BF16 = mybir.dt.bfloat16
F8 = mybir.dt.float8e4
MULT = mybir.AluOpType.mult
ADD = mybir.AluOpType.add
SUB = mybir.AluOpType.subtract
BYP = mybir.AluOpType.bypass
AF = mybir.ActivationFunctionType
DR = mybir.MatmulPerfMode.DoubleRow

# fp8 scale plan for the FFN (power-of-two so dequant is exact):
#   fin_q = S_X * rmsnorm(x+h)      (folded into the rinv broadcast)
#   W1/W3/W2 scaled by S_W host-side (Xavier bound 0.0342*4096 = 140 < 240)
#   ffp_q = S_F * silu(z1) * z3
S_X = 16.0
S_W = 4096.0
S_F = 16.0
K_FFP = S_F / (S_X * S_X * S_W * S_W)   # PSUM(sf)*PSUM(zf3) -> ffp_q
K_SIG = 1.0 / (S_X * S_W)               # PSUM(zf1) -> sigmoid arg
K_OUT = 1.0 / (S_F * S_W)               # PSUM(zo) -> ff output

GROUPS = [[0, 1], [2, 3], [4, 5], [6, 7]]


def build_nc(D, DFF, T, CH=512, BLK=1024, pipe_depth=2):
    """Build the per-core program over T own-tokens. Returns finalized Bacc."""
    kd = D // P            # K-chunks over D
    mf = DFF // P          # m-tiles over DFF
    CHS = [CH] * (T // CH)
    OFFS = np.concatenate([[0], np.cumsum(CHS)[:-1]]).tolist()
    n_ch = len(CHS)
    n_blk = T // BLK
    NS = min(512, BLK)     # matmul/psum free-dim sub-chunk
    nspl = BLK // NS

    nc = bacc.Bacc("TRN2", num_devices=8)
    xt = nc.dram_tensor("xt", (P, kd, T), F32, kind="ExternalInput")
    wg = nc.dram_tensor("wg", (P, kd, D), BF16, kind="ExternalInput")
    wc = nc.dram_tensor("wc", (P, kd, D), BF16, kind="ExternalInput")
    bias = nc.dram_tensor("bias", (P, 3, kd), F32, kind="ExternalInput")
    cmask = nc.dram_tensor("cmask", (P, 1), F32, kind="ExternalInput")
    w1 = nc.dram_tensor("w1", (P, kd, DFF), F8, kind="ExternalInput")
    w3 = nc.dram_tensor("w3", (P, kd, DFF), F8, kind="ExternalInput")
    # repacked host-side so each output-channel block is contiguous
    w2 = nc.dram_tensor("w2", (P, kd, mf, P), F8, kind="ExternalInput")
    y = nc.dram_tensor("y", (P, kd, T), F32, kind="ExternalOutput")

    with TileContext(nc) as tc, ExitStack() as ctx:
        consts = ctx.enter_context(tc.tile_pool(name="consts", bufs=1))
        ones_k = consts.tile([P, 1], F32)
        nc.vector.memset(ones_k[:], 1.0)
        ones_b = consts.tile([1, P], BF16)
        nc.vector.memset(ones_b[:], 1.0)
        sx_b = consts.tile([1, P], BF16)
        nc.vector.memset(sx_b[:], S_X)
        eps_t = consts.tile([1, 1], F32)
        nc.vector.memset(eps_t[:], EPS)
        zero_bf = consts.tile([P, CH], BF16)
        nc.vector.memset(zero_bf[:], 0.0)
        bias_s = consts.tile([P, 3, kd], F32)
        nc.sync.dma_start(bias_s[:], bias[:])
        cmask_s = consts.tile([P, 1], F32)
        nc.sync.dma_start(cmask_s[:], cmask[:])

        dram = ctx.enter_context(tc.tile_pool(name="dram", bufs=1, space="DRAM"))
        xnew_d = dram.tile([P, kd, T], F32)
        cin_d = dram.tile([P, kd, 1], BF16)
        cout_d = dram.tile([2, P, kd, 1], BF16)
        warm_d = dram.tile([P, 1], F32)
        warm_o = dram.tile([2, P, 1], F32)

        # handed to phase 2 in SBUF (x+h itself goes via the f32 DRAM spill,
        # re-loaded per block in phase 2)
        handoff = ctx.enter_context(tc.tile_pool(name="handoff", bufs=1))
        a_all = handoff.tile([P, kd, T], BF16)       # running gate product
        carry_sel = handoff.tile([P, kd, 1], F32)    # mask * partner carry
        hlast = handoff.tile([P, kd], BF16)          # compacted carry column
        rinv_my = handoff.tile([1, T], BF16)

        def norm_reduce(src, rinv, sqpool, npsum, width):
            # 1/rms of src [P, kd, width] over the channel axis -> rinv
            # [1, width]. Squares on ScalarE keep the vector engine free;
            # the partition reduce is a ones-matmul.
            for o in range(0, width, 512):
                w_ = min(512, width - o)
                sl = slice(o, o + w_)
                ssq = npsum.tile([1, 512], F32, name="ssq")[:, :w_]
                for k in range(kd):
                    sq = sqpool.tile([P, 512], F32, name="sq")[:, :w_]
                    nc.scalar.square(sq, src[:, k, sl])
                    nc.tensor.matmul(ssq, ones_k[:], sq,
                                     start=(k == 0), stop=(k == kd - 1))
                # HW-measured max rel err 4e-5 for this LUT
                nc.scalar.activation(rinv[:, sl], ssq,
                                     AF.Abs_reciprocal_sqrt,
                                     bias=eps_t[:], scale=1.0 / D)

        def norm_apply(src, rinv, out, bpsum, width, bvec=None):
            # out = src * broadcast(rinv) (K=1 ones-matmul broadcast);
            # bvec=sx_b folds the fp8 input scale into the broadcast.
            if bvec is None:
                bvec = ones_b
            for o in range(0, width, 512):
                w_ = min(512, width - o)
                sl = slice(o, o + w_)
                rb = bpsum.tile([P, 512], F32, name="rb")[:, :w_]
                nc.tensor.matmul(rb, bvec[:], rinv[:, sl],
                                 start=True, stop=True)
                for k in range(kd):
                    nc.vector.tensor_mul(out[:, k, sl], src[:, k, sl], rb)

        # ---------------- phase 1: gates/cands + scan ----------------
        with (
            tc.tile_pool(name="p1w", bufs=1) as wpool,
            tc.tile_pool(name="p1x", bufs=3) as xpool,
            tc.tile_pool(name="p1hin", bufs=2) as hinpool,
            tc.tile_pool(name="p1sq", bufs=2) as sqpool,
            tc.tile_pool(name="p1s", bufs=2) as spool,
            tc.tile_pool(name="p1scr", bufs=4) as scr,
            tc.tile_pool(name="p1h", bufs=2) as hpool,
            tc.tile_pool(name="p1c", bufs=1) as cpool,
            tc.tile_pool(name="p1np", bufs=1, space="PSUM") as npsum,
            tc.tile_pool(name="p1bp", bufs=1, space="PSUM") as bpsum,
            tc.tile_pool(name="p1zp", bufs=3, space="PSUM") as zpsum,
        ):
            def load_and_norm(c):
                off, w = OFFS[c], CHS[c]
                xt_c = xpool.tile([P, kd, CH], F32, name="xt_c")[:, :, :w]
                for k in range(kd):
                    nc.sync.dma_start(xt_c[:, k, :], xt[:, k, off:off + w])
                hin = hinpool.tile([P, kd, CH], BF16, name="hin")[:, :, :w]
                rinv = spool.tile([1, CH], BF16, name="rinv")[:, :w]
                norm_reduce(xt_c, rinv, sqpool, npsum, w)
                norm_apply(xt_c, rinv, hin, bpsum, w)
                return xt_c, hin

            pipe = [load_and_norm(0)]

            # warm up the collective channel so the real exchange at the end
            # of phase 1 doesn't pay first-touch setup costs
            nc.sync.dma_start(warm_d[:], ones_k[:])
            nc.gpsimd.collective_compute(
                "AllGather", BYP, replica_groups=GROUPS,
                ins=[warm_d[:].opt()], outs=[warm_o[:].opt()])

            wg_s = wpool.tile([P, kd, D], BF16)
            nc.sync.dma_start(wg_s[:], wg[:])
            wc_s = wpool.tile([P, kd, D], BF16)
            nc.sync.dma_start(wc_s[:], wc[:])
            g_tail = []
            if pipe_depth > 1 and n_ch > 1:
                pipe.append(load_and_norm(1))
            h_prev = None
            for c in range(n_ch):
                xt_c, hin = pipe.pop(0)
                # emit chunk c+2's load+norm ahead so the in-order queues
                # keep the PE fed across the chunk boundary.
                if c + pipe_depth < n_ch:
                    pipe.append(load_and_norm(c + pipe_depth))


                off, w = OFFS[c], CHS[c]
                csl = slice(off, off + w)
                h_t = hpool.tile([P, kd, CH], BF16, name="h_t")[:, :, :w]
                for m in range(kd):
                    ms = slice(m * P, (m + 1) * P)
                    zg = zpsum.tile([P, CH], F32, name="zg")[:, :w]
                    zc = zpsum.tile([P, CH], F32, name="zc")[:, :w]
                    for k in range(kd):
                        nc.tensor.matmul(zg, wg_s[:, k, ms], hin[:, k, :],
                                         start=(k == 0), stop=(k == kd - 1))
                    for k in range(kd):
                        nc.tensor.matmul(zc, wc_s[:, k, ms], hin[:, k, :],
                                         start=(k == 0), stop=(k == kd - 1))
                    last = c == n_ch - 1
                    if last:
                        # write the gates straight into a_all: the A-scan
                        # runs in place after the collective trigger, so no
                        # A work sits between scan-h(7) and the carry DMA
                        g_t = a_all[:, m, csl]
                    else:
                        g_t = scr.tile([P, CH], BF16, name="g_t")[:, :w]
                    nc.scalar.activation(g_t, zg, AF.Sigmoid,
                                         bias=bias_s[:, 0, m:m + 1])
                    c_t = scr.tile([P, CH], BF16, name="c_t")[:, :w]
                    nc.scalar.activation(c_t, zc, AF.Tanh,
                                         bias=bias_s[:, 2, m:m + 1])
                    # bn = (g-1)*c = -(1-g)*c in ONE op; the scan uses
                    # op1=subtract so state = g*state - bn
                    b_t = scr.tile([P, CH], BF16, name="b_t")[:, :w]
                    nc.vector.scalar_tensor_tensor(
                        b_t, g_t, 1.0, c_t, op0=SUB, op1=MULT)
                    init_h = (0.0 if h_prev is None
                              else h_prev[:, m, CHS[c - 1] - 1:CHS[c - 1]])
                    nc.vector.tensor_tensor_scan(
                        h_t[:, m, :], g_t, b_t, init_h, op0=MULT, op1=SUB)
                    if c == n_ch - 1:
                        # compact the carry column so the bounce DMA is one
                        # contiguous 16B/partition transfer, not 8 strided
                        # 2B elements (which costs ~20us in descriptors)
                        nc.vector.tensor_copy(hlast[:, m:m + 1],
                                              h_t[:, m, w - 1:w])
                    # running gate product for the cross-half carry fixup
                    # (deferred past the collective trigger for the last
                    # chunk — see above)
                    init_a = 1.0 if c == 0 else a_all[:, m, off - 1:off]
                    if not last:
                        nc.vector.tensor_tensor_scan(
                            a_all[:, m, csl], g_t, zero_bf[:, :w], init_a,
                            op0=MULT, op1=SUB)
                    else:
                        g_tail.append((m, g_t, init_a))
                h_prev = h_t

                if c == n_ch - 1:
                    # ---- carry exchange: fire as soon as h is complete.
                    # The bounce DMAs go through the GpSimd queue: the sync
                    # DMA rings carry the big weight prefetches, and the
                    # collective's completion wait would queue behind them.
                    nc.gpsimd.dma_start(cin_d[:, :, 0], hlast[:])
                    nc.gpsimd.collective_compute(
                        "AllGather", BYP, replica_groups=GROUPS,
                        ins=[cin_d[:].opt()], outs=[cout_d[:].opt()])
                    for m, g_t, init_a in g_tail:
                        # in place: a_all holds the raw gates, the scan
                        # streams element-by-element so out==data0 is safe
                        nc.vector.tensor_tensor_scan(
                            g_t, g_t, zero_bf[:, :w], init_a,
                            op0=MULT, op1=SUB)

                for k in range(kd):
                    # residual x+h_loc on GpSimd; f32 spill (pre-carry-fix)
                    # to DRAM — phase 2 re-loads and carry-fixes it there
                    nc.gpsimd.tensor_add(xt_c[:, k, :], xt_c[:, k, :],
                                         h_t[:, k, :])
                nc.sync.dma_start(xnew_d[:, :, csl], xt_c[:])

            c0 = cpool.tile([P, kd, 1], BF16)
            nc.gpsimd.dma_start(c0[:], cout_d[0])
            nc.vector.scalar_tensor_tensor(
                carry_sel[:], c0[:], cmask_s[:, 0:1], c0[:],
                op0=MULT, op1=BYP)

        # ---------------- phase 2: SwiGLU FFN (fp8 DoubleRow) ----------------
        with (
            tc.tile_pool(name="p2fin", bufs=1) as finpool,
            tc.tile_pool(name="p2w", bufs=1) as wbig,
            tc.tile_pool(name="p2w2", bufs=2) as w2str,
            tc.tile_pool(name="p2ffp", bufs=1) as ffppool,
            tc.tile_pool(name="p2xf", bufs=1) as xfpool,
            tc.tile_pool(name="p2sf", bufs=3) as sfscr,
            tc.tile_pool(name="p2y", bufs=3) as ypool,
            tc.tile_pool(name="p2bp", bufs=1, space="PSUM") as bpsum2,
            tc.tile_pool(name="p2fp", bufs=2, space="PSUM") as fpsum,
            tc.tile_pool(name="p2op", bufs=2, space="PSUM") as opsum,
        ):
            w1s = wbig.tile([P, kd, DFF], F8)
            nc.sync.dma_start(w1s[:], w1[:])
            w3s = wbig.tile([P, kd, DFF], F8)
            nc.sync.dma_start(w3s[:], w3[:])
            for blk in range(n_blk):
                bs = slice(blk * BLK, (blk + 1) * BLK)
                # re-load the block's f32 residual from the spill (the DMA
                # only depends on phase-1's spill, so it flies during the
                # collective window) and carry-fix it in place in f32
                xblk = xfpool.tile([P, kd, BLK], F32)
                nc.sync.dma_start(xblk[:], xnew_d[:, :, bs])
                for o in range(0, BLK, 512):
                    for k in range(kd):
                        nc.vector.scalar_tensor_tensor(
                            xblk[:, k, o:o + 512],
                            a_all[:, k, blk * BLK + o:blk * BLK + o + 512],
                            carry_sel[:, k, :], xblk[:, k, o:o + 512],
                            op0=MULT, op1=ADD)
                norm_reduce(xblk, rinv_my[:, bs], sfscr, bpsum2, BLK)
                fin = finpool.tile([P, kd, BLK], F8)
                norm_apply(xblk, rinv_my[:, bs], fin,
                           bpsum2, BLK, bvec=sx_b)

                ffp = ffppool.tile([P, mf, BLK], F8)
                for mt in range(mf):
                    mts = slice(mt * P, (mt + 1) * P)
                    for h in range(nspl):
                        hs = slice(h * NS, (h + 1) * NS)
                        zf1 = fpsum.tile([P, NS], F32, name="zf1")
                        zf3 = fpsum.tile([P, NS], F32, name="zf3")
                        for k in range(0, kd, 2):
                            nc.tensor.matmul(zf1, w1s[:, k:k + 2, mts],
                                             fin[:, k:k + 2, hs],
                                             start=(k == 0), stop=(k == kd - 2),
                                             perf_mode=DR)
                        for k in range(0, kd, 2):
                            nc.tensor.matmul(zf3, w3s[:, k:k + 2, mts],
                                             fin[:, k:k + 2, hs],
                                             start=(k == 0), stop=(k == kd - 2),
                                             perf_mode=DR)
                        sg = sfscr.tile([P, NS], F32, name="sg")
                        nc.scalar.activation(sg, zf1, AF.Sigmoid, scale=K_SIG)
                        sf = sfscr.tile([P, NS], F32, name="sf")
                        nc.vector.tensor_mul(sf, zf1, sg)
                        nc.vector.scalar_tensor_tensor(
                            ffp[:, mt, hs], sf, K_FFP, zf3,
                            op0=MULT, op1=MULT)

                for m in range(kd):
                    w2_t = w2str.tile([P, mf, P], F8)
                    nc.sync.dma_start(w2_t[:], w2[:, m])
                    for h in range(nspl):
                        ts = slice(blk * BLK + h * NS, blk * BLK + (h + 1) * NS)
                        hs = slice(h * NS, (h + 1) * NS)
                        zo = opsum.tile([P, NS], F32)
                        for k2 in range(0, mf, 2):
                            nc.tensor.matmul(zo, w2_t[:, k2:k2 + 2, :],
                                             ffp[:, k2:k2 + 2, hs],
                                             start=(k2 == 0),
                                             stop=(k2 == mf - 2),
                                             perf_mode=DR)
                        # xblk is already carry-fixed f32: one residual op
                        yt = ypool.tile([P, NS], F32)
                        nc.vector.scalar_tensor_tensor(
                            yt, zo, K_OUT, xblk[:, m, hs], op0=MULT, op1=ADD)
                        nc.sync.dma_start(y[:, m, ts], yt)

    nc.finalize()
    return nc


def _pack_lhsT(w, kd):
    # [K, M] -> [128, K/128, M] with [p, k, m] = w[k*128+p, m]
    K, M = w.shape
    return np.ascontiguousarray(
        w.reshape(kd, P, M).transpose(1, 0, 2)).astype(ml_dtypes.bfloat16)


def _pack_lhsT_f8(w, kd, scale):
    K, M = w.shape
    ws = w * scale
    assert np.abs(ws).max() <= 240.0, f"fp8 overflow: {np.abs(ws).max()}"
    return np.ascontiguousarray(
        ws.reshape(kd, P, M).transpose(1, 0, 2)).astype(ml_dtypes.float8_e4m3)


def _prep_core_inputs(x, Wg, bg, Wc, bc, n1_w, n2_w, W1, W3, W2):
    B, L, D = x.shape
    DFF = W1.shape[1]
    kd, mf = D // P, DFF // P
    T = L // 2

    wg_h = _pack_lhsT(n1_w[:, None] * Wg, kd)
    wc_h = _pack_lhsT(n1_w[:, None] * Wc, kd)
    w1_h = _pack_lhsT_f8(n2_w[:, None] * W1, kd, S_W)
    w3_h = _pack_lhsT_f8(n2_w[:, None] * W3, kd, S_W)
    # [P, mf, D] -> [P, kd, mf, P]: output-channel blocks contiguous
    w2_h = np.ascontiguousarray(
        _pack_lhsT_f8(W2, mf, S_W).reshape(P, mf, kd, P).transpose(0, 2, 1, 3))
    bias_h = np.ascontiguousarray(np.stack(
        [bg.reshape(kd, P).T, -bg.reshape(kd, P).T, bc.reshape(kd, P).T],
        axis=1)).astype(np.float32)

    in_maps = []
    for c in range(8):
        b, s = c // 2, c % 2
        xb = x[b, s * T:(s + 1) * T]
        xt_h = np.ascontiguousarray(
            xb.T.reshape(kd, P, T).transpose(1, 0, 2)).astype(np.float32)
        cmask_h = np.full((P, 1), float(s), np.float32)
        in_maps.append({"xt": xt_h, "wg": wg_h, "wc": wc_h, "bias": bias_h,
                        "cmask": cmask_h,
                        "w1": w1_h, "w3": w3_h, "w2": w2_h})
    return in_maps


_NC_CACHE = {}


def kernel(x, Wg, bg, Wc, bc, n1_w, n2_w, W1, W3, W2, _collect_perf=None):
    from concourse.bass_utils import run_bass_kernel_spmd

    x = np.asarray(x, np.float32)
    B, L, D = x.shape
    DFF = np.asarray(W1).shape[1]
    T = L // 2

    key = (D, DFF, L)
    if key not in _NC_CACHE:
        _NC_CACHE[key] = build_nc(
            D, DFF, T, pipe_depth=int(os.environ.get("K_PIPE", "2")))
    nc = _NC_CACHE[key]

    in_maps = _prep_core_inputs(
        x, *[np.asarray(a, np.float32) for a in
             (Wg, bg, Wc, bc, n1_w, n2_w, W1, W3, W2)])

    res = run_bass_kernel_spmd(nc, in_maps, core_ids=list(range(8)))
    if _collect_perf is not None:
        _collect_perf.append(res)

    kd = D // P
    out = np.empty((B, L, D), np.float32)
    for c in range(8):
        b, s = c // 2, c % 2
        yc = res.results[c]["y"]  # [P, kd, T]
        out[b, s * T:(s + 1) * T] = yc.transpose(2, 1, 0).reshape(T, D)
    return out


# revision 78
# speedup vs baseline: 1.0441x; 1.0441x over previous
"""MinGRU block (RMSNorm -> minGRU scan -> residual -> RMSNorm -> SwiGLU FFN
-> residual) for Trainium2, SPMD over 8 NeuronCores.

Sharding: core c handles batch b=c//2, token-half s=c%2 (2048 tokens each).
Phase 1 (gate/cand matmuls + scan) runs only on the core's own tokens; the
cross-half scan dependency is resolved with the linear-recurrence split
h_true = h_loc + A_loc * carry, where A_loc is the running product of gates
(a second tensor_tensor_scan) and carry = partner's last h, exchanged via a
2KB pairwise AllGather (DRAM bounce). s=0 cores multiply the carry by a 0.0
mask input. The f32 residual spill happens pre-fix; the A*carry term is
re-added to y during phase 2 (bf16 A/carry: the term decays to 0 in ~100
tokens, so bf16 noise on it is negligible).

The FFN runs entirely in fp8e4 (DoubleRow perf mode, 2 k-tiles per PE
instruction = 2x bf16 throughput): weights are pre-scaled by S_W host-side,
the norm output is quantized to fp8 with S_X folded into the rinv broadcast,
and silu(z1)*z3 is quantized to fp8 by the same DVE op that computes it.
All scales are powers of two; dequant folds into the activation scale and
the final residual scalar_tensor_tensor.

Everything on-device is feature-major [D, tokens]: matmuls keep weights
stationary (lhsT tiles [K=128, M=128]) with activations as the moving
operand. RMSNorm's partition-dim reduce/broadcast go through the tensor
engine (ones-vector matmuls); squares run on ScalarE; the residual adds run
on GpSimd; the two scans (h and A), the gate-combine STT, and the bf16
handoff copies run on the DVE. The carry bounce DMAs go through the GpSimd
queue so they don't wait behind weight prefetches on the sync DMA rings, and
a warmup AllGather at program start pays the channel's first-touch cost off
the critical path.
"""

import os
import sys

sys.path.insert(0, "/opt/trn_rl_repo")

from contextlib import ExitStack

import ml_dtypes
import numpy as np

import concourse.bass as bass
import concourse.mybir as mybir
from concourse import bacc
from concourse.tile import TileContext

P = 128
EPS = 1e-6
F32 = mybir.dt.float32
BF16 = mybir.dt.bfloat16
F8 = mybir.dt.float8e4
MULT = mybir.AluOpType.mult
ADD = mybir.AluOpType.add
SUB = mybir.AluOpType.subtract
BYP = mybir.AluOpType.bypass
AF = mybir.ActivationFunctionType
DR = mybir.MatmulPerfMode.DoubleRow

# fp8 scale plan for the FFN (power-of-two so dequant is exact):
#   fin_q = S_X * rmsnorm(x+h)      (folded into the rinv broadcast)
#   W1/W3/W2 scaled by S_W host-side (Xavier bound 0.0342*4096 = 140 < 240)
#   ffp_q = S_F * silu(z1) * z3
S_X = 16.0
S_W = 4096.0
S_F = 16.0
K_FFP = S_F / (S_X * S_X * S_W * S_W)   # PSUM(sf)*PSUM(zf3) -> ffp_q
K_SIG = 1.0 / (S_X * S_W)               # PSUM(zf1) -> sigmoid arg
K_OUT = 1.0 / (S_F * S_W)               # PSUM(zo) -> ff output

GROUPS = [[0, 1], [2, 3], [4, 5], [6, 7]]


def build_nc(D, DFF, T, CH=512, BLK=1024, pipe_depth=2):
    """Build the per-core program over T own-tokens. Returns finalized Bacc."""
    kd = D // P            # K-chunks over D
    mf = DFF // P          # m-tiles over DFF
    CHS = [CH] * (T // CH)
    OFFS = np.concatenate([[0], np.cumsum(CHS)[:-1]]).tolist()
    n_ch = len(CHS)
    n_blk = T // BLK
    NS = min(512, BLK)     # matmul/psum free-dim sub-chunk
    nspl = BLK // NS

    nc = bacc.Bacc("TRN2", num_devices=8)
    xt = nc.dram_tensor("xt", (P, kd, T), F32, kind="ExternalInput")
    wg = nc.dram_tensor("wg", (P, kd, D), BF16, kind="ExternalInput")
    wc = nc.dram_tensor("wc", (P, kd, D), BF16, kind="ExternalInput")
    bias = nc.dram_tensor("bias", (P, 3, kd), F32, kind="ExternalInput")
    cmask = nc.dram_tensor("cmask", (P, 1), F32, kind="ExternalInput")
    w1 = nc.dram_tensor("w1", (P, kd, DFF), F8, kind="ExternalInput")
    w3 = nc.dram_tensor("w3", (P, kd, DFF), F8, kind="ExternalInput")
    # repacked host-side so each output-channel block is contiguous
    w2 = nc.dram_tensor("w2", (P, kd, mf, P), F8, kind="ExternalInput")
    y = nc.dram_tensor("y", (P, kd, T), F32, kind="ExternalOutput")

    with TileContext(nc) as tc, ExitStack() as ctx:
        consts = ctx.enter_context(tc.tile_pool(name="consts", bufs=1))
        ones_k = consts.tile([P, 1], F32)
        nc.vector.memset(ones_k[:], 1.0)
        ones_b = consts.tile([1, P], BF16)
        nc.vector.memset(ones_b[:], 1.0)
        sx_b = consts.tile([1, P], BF16)
        nc.vector.memset(sx_b[:], S_X)
        eps_t = consts.tile([1, 1], F32)
        nc.vector.memset(eps_t[:], EPS)
        zero_bf = consts.tile([P, CH], BF16)
        nc.vector.memset(zero_bf[:], 0.0)
        bias_s = consts.tile([P, 3, kd], F32)
        nc.sync.dma_start(bias_s[:], bias[:])
        cmask_s = consts.tile([P, 1], F32)
        nc.sync.dma_start(cmask_s[:], cmask[:])

        dram = ctx.enter_context(tc.tile_pool(name="dram", bufs=1, space="DRAM"))
        xnew_d = dram.tile([P, kd, T], F32)
        cin_d = dram.tile([P, kd, 1], BF16)
        cout_d = dram.tile([2, P, kd, 1], BF16)
        warm_d = dram.tile([P, 1], F32)
        warm_o = dram.tile([2, P, 1], F32)

        # handed to phase 2 in SBUF
        handoff = ctx.enter_context(tc.tile_pool(name="handoff", bufs=1))
        xnew_bf = handoff.tile([P, kd, T], BF16)
        a_all = handoff.tile([P, kd, T], BF16)       # running gate product
        carry_sel = handoff.tile([P, kd, 1], F32)    # mask * partner carry
        hlast = handoff.tile([P, kd], BF16)          # compacted carry column
        rinv_my = handoff.tile([1, T], BF16)

        def norm_reduce(src, rinv, sqpool, npsum, width):
            # 1/rms of src [P, kd, width] over the channel axis -> rinv
            # [1, width]. Squares on ScalarE keep the vector engine free;
            # the partition reduce is a ones-matmul.
            for o in range(0, width, 512):
                w_ = min(512, width - o)
                sl = slice(o, o + w_)
                ssq = npsum.tile([1, 512], F32, name="ssq")[:, :w_]
                for k in range(kd):
                    sq = sqpool.tile([P, 512], F32, name="sq")[:, :w_]
                    nc.scalar.square(sq, src[:, k, sl])
                    nc.tensor.matmul(ssq, ones_k[:], sq,
                                     start=(k == 0), stop=(k == kd - 1))
                # HW-measured max rel err 4e-5 for this LUT
                nc.scalar.activation(rinv[:, sl], ssq,
                                     AF.Abs_reciprocal_sqrt,
                                     bias=eps_t[:], scale=1.0 / D)

        def norm_apply(src, rinv, out, bpsum, width, bvec=None):
            # out = src * broadcast(rinv) (K=1 ones-matmul broadcast);
            # bvec=sx_b folds the fp8 input scale into the broadcast.
            if bvec is None:
                bvec = ones_b
            for o in range(0, width, 512):
                w_ = min(512, width - o)
                sl = slice(o, o + w_)
                rb = bpsum.tile([P, 512], F32, name="rb")[:, :w_]
                nc.tensor.matmul(rb, bvec[:], rinv[:, sl],
                                 start=True, stop=True)
                for k in range(kd):
                    nc.vector.tensor_mul(out[:, k, sl], src[:, k, sl], rb)

        # ---------------- phase 1: gates/cands + scan ----------------
        with (
            tc.tile_pool(name="p1w", bufs=1) as wpool,
            tc.tile_pool(name="p1x", bufs=3) as xpool,
            tc.tile_pool(name="p1hin", bufs=2) as hinpool,
            tc.tile_pool(name="p1sq", bufs=2) as sqpool,
            tc.tile_pool(name="p1s", bufs=2) as spool,
            tc.tile_pool(name="p1scr", bufs=4) as scr,
            tc.tile_pool(name="p1h", bufs=2) as hpool,
            tc.tile_pool(name="p1c", bufs=1) as cpool,
            tc.tile_pool(name="p1np", bufs=1, space="PSUM") as npsum,
            tc.tile_pool(name="p1bp", bufs=1, space="PSUM") as bpsum,
            tc.tile_pool(name="p1zp", bufs=3, space="PSUM") as zpsum,
        ):
            def load_and_norm(c):
                off, w = OFFS[c], CHS[c]
                xt_c = xpool.tile([P, kd, CH], F32, name="xt_c")[:, :, :w]
                for k in range(kd):
                    nc.sync.dma_start(xt_c[:, k, :], xt[:, k, off:off + w])
                hin = hinpool.tile([P, kd, CH], BF16, name="hin")[:, :, :w]
                rinv = spool.tile([1, CH], BF16, name="rinv")[:, :w]
                norm_reduce(xt_c, rinv, sqpool, npsum, w)
                norm_apply(xt_c, rinv, hin, bpsum, w)
                return xt_c, hin

            pipe = [load_and_norm(0)]

            # warm up the collective channel so the real exchange at the end
            # of phase 1 doesn't pay first-touch setup costs
            nc.sync.dma_start(warm_d[:], ones_k[:])
            nc.gpsimd.collective_compute(
                "AllGather", BYP, replica_groups=GROUPS,
                ins=[warm_d[:].opt()], outs=[warm_o[:].opt()])

            wg_s = wpool.tile([P, kd, D], BF16)
            nc.sync.dma_start(wg_s[:], wg[:])
            wc_s = wpool.tile([P, kd, D], BF16)
            nc.sync.dma_start(wc_s[:], wc[:])
            g_tail = []
            if pipe_depth > 1 and n_ch > 1:
                pipe.append(load_and_norm(1))
            h_prev = None
            for c in range(n_ch):
                xt_c, hin = pipe.pop(0)
                # emit chunk c+2's load+norm ahead so the in-order queues
                # keep the PE fed across the chunk boundary.
                if c + pipe_depth < n_ch:
                    pipe.append(load_and_norm(c + pipe_depth))


                off, w = OFFS[c], CHS[c]
                csl = slice(off, off + w)
                h_t = hpool.tile([P, kd, CH], BF16, name="h_t")[:, :, :w]
                for m in range(kd):
                    ms = slice(m * P, (m + 1) * P)
                    zg = zpsum.tile([P, CH], F32, name="zg")[:, :w]
                    zc = zpsum.tile([P, CH], F32, name="zc")[:, :w]
                    for k in range(kd):
                        nc.tensor.matmul(zg, wg_s[:, k, ms], hin[:, k, :],
                                         start=(k == 0), stop=(k == kd - 1))
                    for k in range(kd):
                        nc.tensor.matmul(zc, wc_s[:, k, ms], hin[:, k, :],
                                         start=(k == 0), stop=(k == kd - 1))
                    last = c == n_ch - 1
                    if last:
                        # write the gates straight into a_all: the A-scan
                        # runs in place after the collective trigger, so no
                        # A work sits between scan-h(7) and the carry DMA
                        g_t = a_all[:, m, csl]
                    else:
                        g_t = scr.tile([P, CH], BF16, name="g_t")[:, :w]
                    nc.scalar.activation(g_t, zg, AF.Sigmoid,
                                         bias=bias_s[:, 0, m:m + 1])
                    c_t = scr.tile([P, CH], BF16, name="c_t")[:, :w]
                    nc.scalar.activation(c_t, zc, AF.Tanh,
                                         bias=bias_s[:, 2, m:m + 1])
                    # bn = (g-1)*c = -(1-g)*c in ONE op; the scan uses
                    # op1=subtract so state = g*state - bn
                    b_t = scr.tile([P, CH], BF16, name="b_t")[:, :w]
                    nc.vector.scalar_tensor_tensor(
                        b_t, g_t, 1.0, c_t, op0=SUB, op1=MULT)
                    init_h = (0.0 if h_prev is None
                              else h_prev[:, m, CHS[c - 1] - 1:CHS[c - 1]])
                    nc.vector.tensor_tensor_scan(
                        h_t[:, m, :], g_t, b_t, init_h, op0=MULT, op1=SUB)
                    if c == n_ch - 1:
                        # compact the carry column so the bounce DMA is one
                        # contiguous 16B/partition transfer, not 8 strided
                        # 2B elements (which costs ~20us in descriptors)
                        nc.vector.tensor_copy(hlast[:, m:m + 1],
                                              h_t[:, m, w - 1:w])
                    # running gate product for the cross-half carry fixup
                    # (deferred past the collective trigger for the last
                    # chunk — see above)
                    init_a = 1.0 if c == 0 else a_all[:, m, off - 1:off]
                    if not last:
                        nc.vector.tensor_tensor_scan(
                            a_all[:, m, csl], g_t, zero_bf[:, :w], init_a,
                            op0=MULT, op1=SUB)
                    else:
                        g_tail.append((m, g_t, init_a))
                h_prev = h_t

                if c == n_ch - 1:
                    # ---- carry exchange: fire as soon as h is complete.
                    # The bounce DMAs go through the GpSimd queue: the sync
                    # DMA rings carry the big weight prefetches, and the
                    # collective's completion wait would queue behind them.
                    nc.gpsimd.dma_start(cin_d[:, :, 0], hlast[:])
                    nc.gpsimd.collective_compute(
                        "AllGather", BYP, replica_groups=GROUPS,
                        ins=[cin_d[:].opt()], outs=[cout_d[:].opt()])
                    for m, g_t, init_a in g_tail:
                        # in place: a_all holds the raw gates, the scan
                        # streams element-by-element so out==data0 is safe
                        nc.vector.tensor_tensor_scan(
                            g_t, g_t, zero_bf[:, :w], init_a,
                            op0=MULT, op1=SUB)

                for k in range(kd):
                    # residual x+h_loc and the bf16 copy both on GpSimd: the
                    # DVE (b_t + two scans + norm-apply muls ~35us/chunk) is
                    # phase 1's busiest engine, GpSimd has slack
                    nc.gpsimd.tensor_add(xnew_bf[:, k, csl], xt_c[:, k, :],
                                         h_t[:, k, :])
                    nc.gpsimd.tensor_add(xt_c[:, k, :], xt_c[:, k, :],
                                         h_t[:, k, :])
                nc.sync.dma_start(xnew_d[:, :, csl], xt_c[:])

            c0 = cpool.tile([P, kd, 1], BF16)
            nc.gpsimd.dma_start(c0[:], cout_d[0])
            nc.vector.scalar_tensor_tensor(
                carry_sel[:], c0[:], cmask_s[:, 0:1], c0[:],
                op0=MULT, op1=BYP)
            # h_true = h_loc + A * carry, folded into the bf16 handoff
            for o in range(0, T, CH):
                csl = slice(o, o + CH)
                for k in range(kd):
                    nc.vector.scalar_tensor_tensor(
                        xnew_bf[:, k, csl], a_all[:, k, csl],
                        carry_sel[:, k, :], xnew_bf[:, k, csl],
                        op0=MULT, op1=ADD)
            # phase-2 norm reduces (post-fix)
            for blk in range(n_blk):
                norm_reduce(xnew_bf[:, :, blk * BLK:(blk + 1) * BLK],
                            rinv_my[:, blk * BLK:(blk + 1) * BLK],
                            sqpool, npsum, BLK)

        # ---------------- phase 2: SwiGLU FFN (fp8 DoubleRow) ----------------
        with (
            tc.tile_pool(name="p2fin", bufs=1) as finpool,
            tc.tile_pool(name="p2w", bufs=1) as wbig,
            tc.tile_pool(name="p2w2", bufs=2) as w2str,
            tc.tile_pool(name="p2ffp", bufs=1) as ffppool,
            tc.tile_pool(name="p2sf", bufs=3) as sfscr,
            tc.tile_pool(name="p2res", bufs=3) as respool,
            tc.tile_pool(name="p2y", bufs=3) as ypool,
            tc.tile_pool(name="p2bp", bufs=1, space="PSUM") as bpsum2,
            tc.tile_pool(name="p2fp", bufs=2, space="PSUM") as fpsum,
            tc.tile_pool(name="p2op", bufs=2, space="PSUM") as opsum,
        ):
            w1s = wbig.tile([P, kd, DFF], F8)
            nc.sync.dma_start(w1s[:], w1[:])
            w3s = wbig.tile([P, kd, DFF], F8)
            nc.sync.dma_start(w3s[:], w3[:])
            for blk in range(n_blk):
                bs = slice(blk * BLK, (blk + 1) * BLK)
                fin = finpool.tile([P, kd, BLK], F8)
                norm_apply(xnew_bf[:, :, bs], rinv_my[:, bs], fin,
                           bpsum2, BLK, bvec=sx_b)

                ffp = ffppool.tile([P, mf, BLK], F8)
                for mt in range(mf):
                    mts = slice(mt * P, (mt + 1) * P)
                    for h in range(nspl):
                        hs = slice(h * NS, (h + 1) * NS)
                        zf1 = fpsum.tile([P, NS], F32, name="zf1")
                        zf3 = fpsum.tile([P, NS], F32, name="zf3")
                        for k in range(0, kd, 2):
                            nc.tensor.matmul(zf1, w1s[:, k:k + 2, mts],
                                             fin[:, k:k + 2, hs],
                                             start=(k == 0), stop=(k == kd - 2),
                                             perf_mode=DR)
                        for k in range(0, kd, 2):
                            nc.tensor.matmul(zf3, w3s[:, k:k + 2, mts],
                                             fin[:, k:k + 2, hs],
                                             start=(k == 0), stop=(k == kd - 2),
                                             perf_mode=DR)
                        sg = sfscr.tile([P, NS], F32, name="sg")
                        nc.scalar.activation(sg, zf1, AF.Sigmoid, scale=K_SIG)
                        sf = sfscr.tile([P, NS], F32, name="sf")
                        nc.vector.tensor_mul(sf, zf1, sg)
                        nc.vector.scalar_tensor_tensor(
                            ffp[:, mt, hs], sf, K_FFP, zf3,
                            op0=MULT, op1=MULT)

                for m in range(kd):
                    w2_t = w2str.tile([P, mf, P], F8)
                    nc.sync.dma_start(w2_t[:], w2[:, m])
                    for h in range(nspl):
                        ts = slice(blk * BLK + h * NS, blk * BLK + (h + 1) * NS)
                        hs = slice(h * NS, (h + 1) * NS)
                        zo = opsum.tile([P, NS], F32)
                        for k2 in range(0, mf, 2):
                            nc.tensor.matmul(zo, w2_t[:, k2:k2 + 2, :],
                                             ffp[:, k2:k2 + 2, hs],
                                             start=(k2 == 0),
                                             stop=(k2 == mf - 2),
                                             perf_mode=DR)
                        xres = respool.tile([P, NS], F32, name="xres")
                        nc.sync.dma_start(xres[:], xnew_d[:, m, ts])
                        yt = ypool.tile([P, NS], F32)
                        nc.vector.scalar_tensor_tensor(
                            yt, zo, K_OUT, xres[:], op0=MULT, op1=ADD)
                        # re-add the carry term the f32 spill missed
                        nc.vector.scalar_tensor_tensor(
                            yt, a_all[:, m, ts], carry_sel[:, m, :], yt,
                            op0=MULT, op1=ADD)
                        nc.sync.dma_start(y[:, m, ts], yt)

    nc.finalize()
    return nc


def _pack_lhsT(w, kd):
    # [K, M] -> [128, K/128, M] with [p, k, m] = w[k*128+p, m]
    K, M = w.shape
    return np.ascontiguousarray(
        w.reshape(kd, P, M).transpose(1, 0, 2)).astype(ml_dtypes.bfloat16)


def _pack_lhsT_f8(w, kd, scale):
    K, M = w.shape
    ws = w * scale
    assert np.abs(ws).max() <= 240.0, f"fp8 overflow: {np.abs(ws).max()}"
    return np.ascontiguousarray(
        ws.reshape(kd, P, M).transpose(1, 0, 2)).astype(ml_dtypes.float8_e4m3)


def _prep_core_inputs(x, Wg, bg, Wc, bc, n1_w, n2_w, W1, W3, W2):
    B, L, D = x.shape
    DFF = W1.shape[1]
    kd, mf = D // P, DFF // P
    T = L // 2

    wg_h = _pack_lhsT(n1_w[:, None] * Wg, kd)
    wc_h = _pack_lhsT(n1_w[:, None] * Wc, kd)
    w1_h = _pack_lhsT_f8(n2_w[:, None] * W1, kd, S_W)
    w3_h = _pack_lhsT_f8(n2_w[:, None] * W3, kd, S_W)
    # [P, mf, D] -> [P, kd, mf, P]: output-channel blocks contiguous
    w2_h = np.ascontiguousarray(
        _pack_lhsT_f8(W2, mf, S_W).reshape(P, mf, kd, P).transpose(0, 2, 1, 3))
    bias_h = np.ascontiguousarray(np.stack(
        [bg.reshape(kd, P).T, -bg.reshape(kd, P).T, bc.reshape(kd, P).T],
        axis=1)).astype(np.float32)

    in_maps = []
    for c in range(8):
        b, s = c // 2, c % 2
        xb = x[b, s * T:(s + 1) * T]
        xt_h = np.ascontiguousarray(
            xb.T.reshape(kd, P, T).transpose(1, 0, 2)).astype(np.float32)
        cmask_h = np.full((P, 1), float(s), np.float32)
        in_maps.append({"xt": xt_h, "wg": wg_h, "wc": wc_h, "bias": bias_h,
                        "cmask": cmask_h,
                        "w1": w1_h, "w3": w3_h, "w2": w2_h})
    return in_maps


_NC_CACHE = {}


def kernel(x, Wg, bg, Wc, bc, n1_w, n2_w, W1, W3, W2, _collect_perf=None):
    from concourse.bass_utils import run_bass_kernel_spmd

    x = np.asarray(x, np.float32)
    B, L, D = x.shape
    DFF = np.asarray(W1).shape[1]
    T = L // 2

    key = (D, DFF, L)
    if key not in _NC_CACHE:
        _NC_CACHE[key] = build_nc(
            D, DFF, T, pipe_depth=int(os.environ.get("K_PIPE", "2")))
    nc = _NC_CACHE[key]

    in_maps = _prep_core_inputs(
        x, *[np.asarray(a, np.float32) for a in
             (Wg, bg, Wc, bc, n1_w, n2_w, W1, W3, W2)])

    res = run_bass_kernel_spmd(nc, in_maps, core_ids=list(range(8)))
    if _collect_perf is not None:
        _collect_perf.append(res)

    kd = D // P
    out = np.empty((B, L, D), np.float32)
    for c in range(8):
        b, s = c // 2, c % 2
        yc = res.results[c]["y"]  # [P, kd, T]
        out[b, s * T:(s + 1) * T] = yc.transpose(2, 1, 0).reshape(T, D)
    return out


# revision 81
# speedup vs baseline: 1.0581x; 1.0134x over previous
"""MinGRU block (RMSNorm -> minGRU scan -> residual -> RMSNorm -> SwiGLU FFN
-> residual) for Trainium2, SPMD over 8 NeuronCores.

Sharding: core c handles batch b=c//2, token-half s=c%2 (2048 tokens each).
Phase 1 (gate/cand matmuls + scan) runs only on the core's own tokens; the
cross-half scan dependency is resolved with the linear-recurrence split
h_true = h_loc + A_loc * carry, where A_loc is the running product of gates
(a second tensor_tensor_scan) and carry = partner's last h, exchanged via a
2KB pairwise AllGather (DRAM bounce). s=0 cores multiply the carry by a 0.0
mask input. The f32 residual spill happens pre-fix; the A*carry term is
re-added to y during phase 2 (bf16 A/carry: the term decays to 0 in ~100
tokens, so bf16 noise on it is negligible).

The FFN runs entirely in fp8e4 (DoubleRow perf mode, 2 k-tiles per PE
instruction = 2x bf16 throughput): weights are pre-scaled by S_W host-side,
the norm output is quantized to fp8 with S_X folded into the rinv broadcast,
and silu(z1)*z3 is quantized to fp8 by the same DVE op that computes it.
All scales are powers of two; dequant folds into the activation scale and
the final residual scalar_tensor_tensor.

Everything on-device is feature-major [D, tokens]: matmuls keep weights
stationary (lhsT tiles [K=128, M=128]) with activations as the moving
operand. RMSNorm's partition-dim reduce/broadcast go through the tensor
engine (ones-vector matmuls); squares run on ScalarE; the residual adds run
on GpSimd; the two scans (h and A), the gate-combine STT, and the bf16
handoff copies run on the DVE. The carry bounce DMAs go through the GpSimd
queue so they don't wait behind weight prefetches on the sync DMA rings, and
a warmup AllGather at program start pays the channel's first-touch cost off
the critical path.
"""

import os
import sys

sys.path.insert(0, "/opt/trn_rl_repo")

from contextlib import ExitStack

import ml_dtypes
import numpy as np

import concourse.bass as bass
import concourse.mybir as mybir
from concourse import bacc
from concourse.tile import TileContext

P = 128
EPS = 1e-6
F32 = mybir.dt.float32
BF16 = mybir.dt.bfloat16
F8 = mybir.dt.float8e4
MULT = mybir.AluOpType.mult
ADD = mybir.AluOpType.add
SUB = mybir.AluOpType.subtract
BYP = mybir.AluOpType.bypass
AF = mybir.ActivationFunctionType
DR = mybir.MatmulPerfMode.DoubleRow

# fp8 scale plan for the FFN (power-of-two so dequant is exact):
#   fin_q = S_X * rmsnorm(x+h)      (folded into the rinv broadcast)
#   W1/W3/W2 scaled by S_W host-side (Xavier bound 0.0342*4096 = 140 < 240)
#   ffp_q = S_F * silu(z1) * z3
S_X = 16.0
S_W = 4096.0
S_F = 16.0
K_SIG = 1.0 / (S_X * S_W)               # PSUM(zf1) -> silu arg
K_FFP = S_F * K_SIG                     # silu(z1)*PSUM(zf3) -> ffp_q
K_OUT = 1.0 / (S_F * S_W)               # PSUM(zo) -> ff output

GROUPS = [[0, 1], [2, 3], [4, 5], [6, 7]]


def build_nc(D, DFF, T, CH=512, BLK=1024, pipe_depth=2):
    """Build the per-core program over T own-tokens. Returns finalized Bacc."""
    kd = D // P            # K-chunks over D
    mf = DFF // P          # m-tiles over DFF
    CHS = [CH] * (T // CH)
    OFFS = np.concatenate([[0], np.cumsum(CHS)[:-1]]).tolist()
    n_ch = len(CHS)
    n_blk = T // BLK
    NS = min(512, BLK)     # matmul/psum free-dim sub-chunk
    nspl = BLK // NS

    nc = bacc.Bacc("TRN2", num_devices=8)
    xt = nc.dram_tensor("xt", (P, kd, T), F32, kind="ExternalInput")
    wg = nc.dram_tensor("wg", (P, kd, D), BF16, kind="ExternalInput")
    wc = nc.dram_tensor("wc", (P, kd, D), BF16, kind="ExternalInput")
    bias = nc.dram_tensor("bias", (P, 3, kd), F32, kind="ExternalInput")
    cmask = nc.dram_tensor("cmask", (P, 1), F32, kind="ExternalInput")
    w1 = nc.dram_tensor("w1", (P, kd, DFF), F8, kind="ExternalInput")
    w3 = nc.dram_tensor("w3", (P, kd, DFF), F8, kind="ExternalInput")
    # repacked host-side so each output-channel block is contiguous
    w2 = nc.dram_tensor("w2", (P, kd, mf, P), F8, kind="ExternalInput")
    y = nc.dram_tensor("y", (P, kd, T), F32, kind="ExternalOutput")

    with TileContext(nc) as tc, ExitStack() as ctx:
        consts = ctx.enter_context(tc.tile_pool(name="consts", bufs=1))
        ones_k = consts.tile([P, 1], F32)
        nc.vector.memset(ones_k[:], 1.0)
        ones_b = consts.tile([1, P], BF16)
        nc.vector.memset(ones_b[:], 1.0)
        sx_b = consts.tile([1, P], BF16)
        nc.vector.memset(sx_b[:], S_X)
        eps_t = consts.tile([1, 1], F32)
        nc.vector.memset(eps_t[:], EPS)
        zero_bf = consts.tile([P, CH], BF16)
        nc.vector.memset(zero_bf[:], 0.0)
        bias_s = consts.tile([P, 3, kd], F32)
        nc.sync.dma_start(bias_s[:], bias[:])
        cmask_s = consts.tile([P, 1], F32)
        nc.sync.dma_start(cmask_s[:], cmask[:])

        dram = ctx.enter_context(tc.tile_pool(name="dram", bufs=1, space="DRAM"))
        xnew_d = dram.tile([P, kd, T], F32)
        cin_d = dram.tile([P, kd, 1], BF16)
        cout_d = dram.tile([2, P, kd, 1], BF16)
        warm_d = dram.tile([P, 1], F32)
        warm_o = dram.tile([2, P, 1], F32)

        # handed to phase 2 in SBUF
        handoff = ctx.enter_context(tc.tile_pool(name="handoff", bufs=1))
        xnew_bf = handoff.tile([P, kd, T], BF16)
        a_all = handoff.tile([P, kd, T], BF16)       # running gate product
        carry_sel = handoff.tile([P, kd, 1], F32)    # mask * partner carry
        hlast = handoff.tile([P, kd], BF16)          # compacted carry column
        rinv_my = handoff.tile([1, T], BF16)

        def norm_reduce(src, rinv, sqpool, npsum, width):
            # 1/rms of src [P, kd, width] over the channel axis -> rinv
            # [1, width]. Squares on ScalarE keep the vector engine free;
            # the partition reduce is a ones-matmul.
            for o in range(0, width, 512):
                w_ = min(512, width - o)
                sl = slice(o, o + w_)
                ssq = npsum.tile([1, 512], F32, name="ssq")[:, :w_]
                for k in range(kd):
                    sq = sqpool.tile([P, 512], F32, name="sq")[:, :w_]
                    nc.scalar.square(sq, src[:, k, sl])
                    nc.tensor.matmul(ssq, ones_k[:], sq,
                                     start=(k == 0), stop=(k == kd - 1))
                # HW-measured max rel err 4e-5 for this LUT
                nc.scalar.activation(rinv[:, sl], ssq,
                                     AF.Abs_reciprocal_sqrt,
                                     bias=eps_t[:], scale=1.0 / D)

        def norm_apply(src, rinv, out, bpsum, width, bvec=None):
            # out = src * broadcast(rinv) (K=1 ones-matmul broadcast);
            # bvec=sx_b folds the fp8 input scale into the broadcast.
            if bvec is None:
                bvec = ones_b
            for o in range(0, width, 512):
                w_ = min(512, width - o)
                sl = slice(o, o + w_)
                rb = bpsum.tile([P, 512], F32, name="rb")[:, :w_]
                nc.tensor.matmul(rb, bvec[:], rinv[:, sl],
                                 start=True, stop=True)
                for k in range(kd):
                    nc.vector.tensor_mul(out[:, k, sl], src[:, k, sl], rb)

        # ---------------- phase 1: gates/cands + scan ----------------
        with (
            tc.tile_pool(name="p1w", bufs=1) as wpool,
            tc.tile_pool(name="p1x", bufs=3) as xpool,
            tc.tile_pool(name="p1hin", bufs=2) as hinpool,
            tc.tile_pool(name="p1sq", bufs=2) as sqpool,
            tc.tile_pool(name="p1s", bufs=2) as spool,
            tc.tile_pool(name="p1scr", bufs=4) as scr,
            tc.tile_pool(name="p1h", bufs=2) as hpool,
            tc.tile_pool(name="p1c", bufs=1) as cpool,
            tc.tile_pool(name="p1np", bufs=1, space="PSUM") as npsum,
            tc.tile_pool(name="p1bp", bufs=1, space="PSUM") as bpsum,
            tc.tile_pool(name="p1zp", bufs=3, space="PSUM") as zpsum,
        ):
            def load_and_norm(c):
                off, w = OFFS[c], CHS[c]
                xt_c = xpool.tile([P, kd, CH], F32, name="xt_c")[:, :, :w]
                for k in range(kd):
                    nc.sync.dma_start(xt_c[:, k, :], xt[:, k, off:off + w])
                hin = hinpool.tile([P, kd, CH], BF16, name="hin")[:, :, :w]
                rinv = spool.tile([1, CH], BF16, name="rinv")[:, :w]
                norm_reduce(xt_c, rinv, sqpool, npsum, w)
                norm_apply(xt_c, rinv, hin, bpsum, w)
                return xt_c, hin

            pipe = [load_and_norm(0)]

            # warm up the collective channel so the real exchange at the end
            # of phase 1 doesn't pay first-touch setup costs
            nc.sync.dma_start(warm_d[:], ones_k[:])
            nc.gpsimd.collective_compute(
                "AllGather", BYP, replica_groups=GROUPS,
                ins=[warm_d[:].opt()], outs=[warm_o[:].opt()])

            wg_s = wpool.tile([P, kd, D], BF16)
            nc.sync.dma_start(wg_s[:], wg[:])
            wc_s = wpool.tile([P, kd, D], BF16)
            nc.sync.dma_start(wc_s[:], wc[:])
            g_tail = []
            if pipe_depth > 1 and n_ch > 1:
                pipe.append(load_and_norm(1))
            h_prev = None
            for c in range(n_ch):
                xt_c, hin = pipe.pop(0)
                # emit chunk c+2's load+norm ahead so the in-order queues
                # keep the PE fed across the chunk boundary.
                if c + pipe_depth < n_ch:
                    pipe.append(load_and_norm(c + pipe_depth))


                off, w = OFFS[c], CHS[c]
                csl = slice(off, off + w)
                h_t = hpool.tile([P, kd, CH], BF16, name="h_t")[:, :, :w]
                for m in range(kd):
                    ms = slice(m * P, (m + 1) * P)
                    zg = zpsum.tile([P, CH], F32, name="zg")[:, :w]
                    zc = zpsum.tile([P, CH], F32, name="zc")[:, :w]
                    for k in range(kd):
                        nc.tensor.matmul(zg, wg_s[:, k, ms], hin[:, k, :],
                                         start=(k == 0), stop=(k == kd - 1))
                    for k in range(kd):
                        nc.tensor.matmul(zc, wc_s[:, k, ms], hin[:, k, :],
                                         start=(k == 0), stop=(k == kd - 1))
                    last = c == n_ch - 1
                    if last:
                        # write the gates straight into a_all: the A-scan
                        # runs in place after the collective trigger, so no
                        # A work sits between scan-h(7) and the carry DMA
                        g_t = a_all[:, m, csl]
                    else:
                        g_t = scr.tile([P, CH], BF16, name="g_t")[:, :w]
                    nc.scalar.activation(g_t, zg, AF.Sigmoid,
                                         bias=bias_s[:, 0, m:m + 1])
                    c_t = scr.tile([P, CH], BF16, name="c_t")[:, :w]
                    nc.scalar.activation(c_t, zc, AF.Tanh,
                                         bias=bias_s[:, 2, m:m + 1])
                    # bn = (g-1)*c = -(1-g)*c in ONE op; the scan uses
                    # op1=subtract so state = g*state - bn
                    b_t = scr.tile([P, CH], BF16, name="b_t")[:, :w]
                    nc.vector.scalar_tensor_tensor(
                        b_t, g_t, 1.0, c_t, op0=SUB, op1=MULT)
                    init_h = (0.0 if h_prev is None
                              else h_prev[:, m, CHS[c - 1] - 1:CHS[c - 1]])
                    nc.vector.tensor_tensor_scan(
                        h_t[:, m, :], g_t, b_t, init_h, op0=MULT, op1=SUB)
                    if c == n_ch - 1:
                        # compact the carry column so the bounce DMA is one
                        # contiguous 16B/partition transfer, not 8 strided
                        # 2B elements (which costs ~20us in descriptors)
                        nc.vector.tensor_copy(hlast[:, m:m + 1],
                                              h_t[:, m, w - 1:w])
                    # running gate product for the cross-half carry fixup
                    # (deferred past the collective trigger for the last
                    # chunk — see above)
                    init_a = 1.0 if c == 0 else a_all[:, m, off - 1:off]
                    if not last:
                        nc.vector.tensor_tensor_scan(
                            a_all[:, m, csl], g_t, zero_bf[:, :w], init_a,
                            op0=MULT, op1=SUB)
                    else:
                        g_tail.append((m, g_t, init_a))
                h_prev = h_t

                if c == n_ch - 1:
                    # ---- carry exchange: fire as soon as h is complete.
                    # The bounce DMAs go through the GpSimd queue: the sync
                    # DMA rings carry the big weight prefetches, and the
                    # collective's completion wait would queue behind them.
                    nc.gpsimd.dma_start(cin_d[:, :, 0], hlast[:])
                    nc.gpsimd.collective_compute(
                        "AllGather", BYP, replica_groups=GROUPS,
                        ins=[cin_d[:].opt()], outs=[cout_d[:].opt()])
                    for m, g_t, init_a in g_tail:
                        # in place: a_all holds the raw gates, the scan
                        # streams element-by-element so out==data0 is safe
                        nc.vector.tensor_tensor_scan(
                            g_t, g_t, zero_bf[:, :w], init_a,
                            op0=MULT, op1=SUB)

                for k in range(kd):
                    # residual x+h_loc on GpSimd; bf16 copy for phase 2's
                    # norm off DVE; f32 spill (pre-carry-fix) to DRAM
                    nc.gpsimd.tensor_add(xt_c[:, k, :], xt_c[:, k, :],
                                         h_t[:, k, :])
                    nc.vector.tensor_copy(xnew_bf[:, k, csl], xt_c[:, k, :])
                nc.sync.dma_start(xnew_d[:, :, csl], xt_c[:])

            c0 = cpool.tile([P, kd, 1], BF16)
            nc.gpsimd.dma_start(c0[:], cout_d[0])
            nc.vector.scalar_tensor_tensor(
                carry_sel[:], c0[:], cmask_s[:, 0:1], c0[:],
                op0=MULT, op1=BYP)
            # h_true = h_loc + A * carry, folded into the bf16 handoff
            for o in range(0, T, CH):
                csl = slice(o, o + CH)
                for k in range(kd):
                    nc.vector.scalar_tensor_tensor(
                        xnew_bf[:, k, csl], a_all[:, k, csl],
                        carry_sel[:, k, :], xnew_bf[:, k, csl],
                        op0=MULT, op1=ADD)
            # phase-2 norm reduces (post-fix)
            for blk in range(n_blk):
                norm_reduce(xnew_bf[:, :, blk * BLK:(blk + 1) * BLK],
                            rinv_my[:, blk * BLK:(blk + 1) * BLK],
                            sqpool, npsum, BLK)

        # ---------------- phase 2: SwiGLU FFN (fp8 DoubleRow) ----------------
        with (
            tc.tile_pool(name="p2fin", bufs=1) as finpool,
            tc.tile_pool(name="p2w", bufs=1) as wbig,
            tc.tile_pool(name="p2w2", bufs=2) as w2str,
            tc.tile_pool(name="p2ffp", bufs=1) as ffppool,
            tc.tile_pool(name="p2sf", bufs=3) as sfscr,
            tc.tile_pool(name="p2res", bufs=3) as respool,
            tc.tile_pool(name="p2y", bufs=3) as ypool,
            tc.tile_pool(name="p2bp", bufs=1, space="PSUM") as bpsum2,
            tc.tile_pool(name="p2fp", bufs=2, space="PSUM") as fpsum,
            tc.tile_pool(name="p2op", bufs=2, space="PSUM") as opsum,
        ):
            w1s = wbig.tile([P, kd, DFF], F8)
            nc.sync.dma_start(w1s[:], w1[:])
            w3s = wbig.tile([P, kd, DFF], F8)
            nc.sync.dma_start(w3s[:], w3[:])
            for blk in range(n_blk):
                bs = slice(blk * BLK, (blk + 1) * BLK)
                fin = finpool.tile([P, kd, BLK], F8)
                norm_apply(xnew_bf[:, :, bs], rinv_my[:, bs], fin,
                           bpsum2, BLK, bvec=sx_b)

                ffp = ffppool.tile([P, mf, BLK], F8)
                for mt in range(mf):
                    mts = slice(mt * P, (mt + 1) * P)
                    for h in range(nspl):
                        hs = slice(h * NS, (h + 1) * NS)
                        zf1 = fpsum.tile([P, NS], F32, name="zf1")
                        zf3 = fpsum.tile([P, NS], F32, name="zf3")
                        for k in range(0, kd, 2):
                            nc.tensor.matmul(zf1, w1s[:, k:k + 2, mts],
                                             fin[:, k:k + 2, hs],
                                             start=(k == 0), stop=(k == kd - 2),
                                             perf_mode=DR)
                        for k in range(0, kd, 2):
                            nc.tensor.matmul(zf3, w3s[:, k:k + 2, mts],
                                             fin[:, k:k + 2, hs],
                                             start=(k == 0), stop=(k == kd - 2),
                                             perf_mode=DR)
                        # fused silu on the ACT engine: frees the DVE mul and
                        # releases zf1's PSUM bank one op earlier
                        sf = sfscr.tile([P, NS], F32, name="sf")
                        nc.scalar.activation(sf, zf1, AF.Silu, scale=K_SIG)
                        nc.vector.scalar_tensor_tensor(
                            ffp[:, mt, hs], sf, K_FFP, zf3,
                            op0=MULT, op1=MULT)

                for m in range(kd):
                    w2_t = w2str.tile([P, mf, P], F8)
                    nc.sync.dma_start(w2_t[:], w2[:, m])
                    for h in range(nspl):
                        ts = slice(blk * BLK + h * NS, blk * BLK + (h + 1) * NS)
                        hs = slice(h * NS, (h + 1) * NS)
                        zo = opsum.tile([P, NS], F32)
                        for k2 in range(0, mf, 2):
                            nc.tensor.matmul(zo, w2_t[:, k2:k2 + 2, :],
                                             ffp[:, k2:k2 + 2, hs],
                                             start=(k2 == 0),
                                             stop=(k2 == mf - 2),
                                             perf_mode=DR)
                        xres = respool.tile([P, NS], F32, name="xres")
                        nc.sync.dma_start(xres[:], xnew_d[:, m, ts])
                        yt = ypool.tile([P, NS], F32)
                        nc.vector.scalar_tensor_tensor(
                            yt, zo, K_OUT, xres[:], op0=MULT, op1=ADD)
                        # re-add the carry term the f32 spill missed
                        nc.vector.scalar_tensor_tensor(
                            yt, a_all[:, m, ts], carry_sel[:, m, :], yt,
                            op0=MULT, op1=ADD)
                        nc.sync.dma_start(y[:, m, ts], yt)

    nc.finalize()
    return nc


def _pack_lhsT(w, kd):
    # [K, M] -> [128, K/128, M] with [p, k, m] = w[k*128+p, m]
    K, M = w.shape
    return np.ascontiguousarray(
        w.reshape(kd, P, M).transpose(1, 0, 2)).astype(ml_dtypes.bfloat16)


def _pack_lhsT_f8(w, kd, scale):
    K, M = w.shape
    ws = w * scale
    assert np.abs(ws).max() <= 240.0, f"fp8 overflow: {np.abs(ws).max()}"
    return np.ascontiguousarray(
        ws.reshape(kd, P, M).transpose(1, 0, 2)).astype(ml_dtypes.float8_e4m3)


def _prep_core_inputs(x, Wg, bg, Wc, bc, n1_w, n2_w, W1, W3, W2):
    B, L, D = x.shape
    DFF = W1.shape[1]
    kd, mf = D // P, DFF // P
    T = L // 2

    wg_h = _pack_lhsT(n1_w[:, None] * Wg, kd)
    wc_h = _pack_lhsT(n1_w[:, None] * Wc, kd)
    w1_h = _pack_lhsT_f8(n2_w[:, None] * W1, kd, S_W)
    w3_h = _pack_lhsT_f8(n2_w[:, None] * W3, kd, S_W)
    # [P, mf, D] -> [P, kd, mf, P]: output-channel blocks contiguous
    w2_h = np.ascontiguousarray(
        _pack_lhsT_f8(W2, mf, S_W).reshape(P, mf, kd, P).transpose(0, 2, 1, 3))
    bias_h = np.ascontiguousarray(np.stack(
        [bg.reshape(kd, P).T, -bg.reshape(kd, P).T, bc.reshape(kd, P).T],
        axis=1)).astype(np.float32)

    in_maps = []
    for c in range(8):
        b, s = c // 2, c % 2
        xb = x[b, s * T:(s + 1) * T]
        xt_h = np.ascontiguousarray(
            xb.T.reshape(kd, P, T).transpose(1, 0, 2)).astype(np.float32)
        cmask_h = np.full((P, 1), float(s), np.float32)
        in_maps.append({"xt": xt_h, "wg": wg_h, "wc": wc_h, "bias": bias_h,
                        "cmask": cmask_h,
                        "w1": w1_h, "w3": w3_h, "w2": w2_h})
    return in_maps


_NC_CACHE = {}


def kernel(x, Wg, bg, Wc, bc, n1_w, n2_w, W1, W3, W2, _collect_perf=None):
    from concourse.bass_utils import run_bass_kernel_spmd

    x = np.asarray(x, np.float32)
    B, L, D = x.shape
    DFF = np.asarray(W1).shape[1]
    T = L // 2

    key = (D, DFF, L)
    if key not in _NC_CACHE:
        _NC_CACHE[key] = build_nc(
            D, DFF, T, pipe_depth=int(os.environ.get("K_PIPE", "2")))
    nc = _NC_CACHE[key]

    in_maps = _prep_core_inputs(
        x, *[np.asarray(a, np.float32) for a in
             (Wg, bg, Wc, bc, n1_w, n2_w, W1, W3, W2)])

    res = run_bass_kernel_spmd(nc, in_maps, core_ids=list(range(8)))
    if _collect_perf is not None:
        _collect_perf.append(res)

    kd = D // P
    out = np.empty((B, L, D), np.float32)
    for c in range(8):
        b, s = c // 2, c % 2
        yc = res.results[c]["y"]  # [P, kd, T]
        out[b, s * T:(s + 1) * T] = yc.transpose(2, 1, 0).reshape(T, D)
    return out
